# revision 1
# baseline (speedup 1.0000x reference)
"""AttentionBlock Trainium2 kernel.

Problem: B=16, C=256, H=W=32 (N=1024 pixels), GroupNorm(8) -> 1x1-conv QKV ->
softmax attention over pixels -> 1x1-conv proj -> +residual.

Sharding: data-parallel over batch across 8 NeuronCores (2 batch elems/core),
weights replicated.  Weights are pre-transposed on the host so every matmul
operand DMAs contiguously.

Device layout per batch element (channels on partitions):
  x                  : [C=256, N=1024] fp32 (residual + groupnorm stats)
  xn, q, k           : [C, N] float32r (full-rate PE matmuls at free>=256)
  v_aug              : [N, C+1] f32r (pixels on partitions; col 256 = ones so
                       the P@V matmul also produces the softmax row-sums)
  S^T = (q^T k)^T    : [m=1024, n=1024] exp'd by ScalarE straight out of PSUM
                       (no max subtraction: |S| <~ 8 is safe in fp32)
  att^T = P@V        : [n, C] then PE-transposed to [C, n] for the proj.
Residual + proj bias are fused into the PSUM->SBUF move with one
scalar_tensor_tensor op, keeping the residual exact fp32.
"""

from contextlib import ExitStack

import numpy as np

import concourse.bass as bass
import concourse.tile as tile
from concourse import bacc, mybir
from concourse._compat import with_exitstack
from concourse.bass_utils import run_bass_kernel_spmd

# Problem constants (hardcoded per contract)
B, C, H, W = 16, 256, 32, 32
N = H * W            # 1024 pixels
G = 8                # groups
GS = C // G          # 32 channels / group
NCORES = 8
BPC = B // NCORES    # batch elems per core
EPS = 1e-5
P = 128              # partitions
KT = C // P          # 2 c-tiles
NB = N // P          # 8 pixel blocks of 128
NF = N // 512        # 2 free chunks of 512
F32 = mybir.dt.float32
F32R = mybir.dt.float32r
BF16 = mybir.dt.bfloat16
AF = mybir.ActivationFunctionType
OP = mybir.AluOpType


@with_exitstack
def attn_tile_kernel(
    ctx: ExitStack,
    tc: tile.TileContext,
    out_d,
    x_d,
    gamma_d,
    beta_d,
    wqkvT_d,
    bqkv_d,
    wprojT_d,
    bproj_d,
    ident_d,
    gmask_d,
    gmask2_d,
    has_qkv_bias: bool,
    has_proj_bias: bool,
    stop_stage: int = 99,
):
    nc = tc.nc

    consts = ctx.enter_context(tc.tile_pool(name="consts", bufs=1))
    gn = ctx.enter_context(tc.tile_pool(name="gn", bufs=2))
    big = ctx.enter_context(tc.tile_pool(name="big", bufs=2))
    es = ctx.enter_context(tc.tile_pool(name="es", bufs=2))
    # PSUM budget (8 banks), split by draining engine so phases don't
    # block each other on slots:
    #   mm  = 2x [128,1024] S-score psums, drained by ACT exp  (4 banks)
    #   qp  = 2x [128,512] qkv/proj psums, drained by DVE      (2 banks)
    #   pv  = 2x [128,264] PV accum + transposes + gn stats    (2 banks)
    ps512 = ctx.enter_context(tc.tile_pool(name="ps512", bufs=2, space="PSUM"))
    psqp = ctx.enter_context(tc.tile_pool(name="psqp", bufs=2, space="PSUM"))
    pspv = ctx.enter_context(tc.tile_pool(name="pspv", bufs=2, space="PSUM"))
    pssm = pspv

    state = {}

    # batch 0's input DMA goes first: it gates the groupnorm -> QKV critical
    # path, while weights aren't needed until the first matmul ~10us later
    xt0 = big.tile([P, KT, N], F32, tag="xt")
    for kt in range(KT):
        for sub in range(2):
            nc.sync.dma_start(
                xt0[:, kt, sub * 512:(sub + 1) * 512],
                x_d[0, kt * P:(kt + 1) * P, sub * 512:(sub + 1) * 512])
    state[0] = {"xt": xt0}

    # ---- constants / weights (loaded once, replicated across cores) ----
    wqkv_t = consts.tile([P, KT, 3 * C], F32R)
    for kt in range(KT):
        nc.sync.dma_start(wqkv_t[:, kt, :],
                          wqkvT_d[kt * P:(kt + 1) * P, :].bitcast(F32R))
    wproj_t = consts.tile([P, KT, C], F32)
    for kt in range(KT):
        nc.sync.dma_start(wproj_t[:, kt, :], wprojT_d[kt * P:(kt + 1) * P, :])
    ident_f = consts.tile([P, P], F32)
    nc.sync.dma_start(ident_f[:], ident_d.ap())
    ident_t = consts.tile([P, P], BF16)
    nc.vector.tensor_copy(ident_t[:], ident_f[:])
    wproj_b = consts.tile([P, KT, C], BF16)
    for kt in range(KT):
        nc.vector.tensor_copy(wproj_b[:, kt, :], wproj_t[:, kt, :])
    gmask_t = consts.tile([P, 4], F32)
    nc.sync.dma_start(gmask_t[:], gmask_d.ap())
    gmask2_t = consts.tile([4, P], F32)
    nc.sync.dma_start(gmask2_t[:], gmask2_d.ap())
    gamma_t = consts.tile([P, KT], F32)
    beta_t = consts.tile([P, KT], F32)
    for kt in range(KT):
        nc.sync.dma_start(gamma_t[:, kt:kt + 1], gamma_d[kt * P:(kt + 1) * P, :])
        nc.sync.dma_start(beta_t[:, kt:kt + 1], beta_d[kt * P:(kt + 1) * P, :])
    eps_t = consts.tile([P, 1], F32)
    nc.vector.memset(eps_t, EPS)
    one_t = consts.tile([P, 1], F32)
    nc.vector.memset(one_t, 1.0)
    # q/k biases as per-partition columns (q: rows 0..255, k: 256..511)
    if has_qkv_bias:
        bqk_t = consts.tile([P, 2 * KT], F32)
        for j in range(2 * KT):
            nc.sync.dma_start(bqk_t[:, j:j + 1], bqkv_d[j * P:(j + 1) * P, :])
        # v bias lives on the free axis of v_aug -> needs a row layout + ones col
        bv_row = consts.tile([1, C], F32R)
        nc.sync.dma_start(
            bv_row[:],
            bqkv_d[2 * C:3 * C, :].rearrange("c one -> one c").bitcast(F32R))
        ones_row_f32 = consts.tile([1, P], F32)
        nc.vector.memset(ones_row_f32, 1.0)
        ones_col = consts.tile([1, P], F32R)
        nc.vector.tensor_copy(ones_col[:], ones_row_f32[:])
    if has_proj_bias:
        bproj_t = consts.tile([P, KT], F32)
        for kt in range(KT):
            nc.sync.dma_start(bproj_t[:, kt:kt + 1], bproj_d[kt * P:(kt + 1) * P, :])


    def phase_load_gn(b):
        # ---- load x + group norm -> xn ----
        if b in state:
            xt = state[b]["xt"]
        else:
            xt = big.tile([P, KT, N], F32, tag="xt")
            for kt in range(KT):
                nc.sync.dma_start(xt[:, kt, :], x_d[b, kt * P:(kt + 1) * P, :])
        stats = gn.tile([P, KT, 2, 6], F32, tag="stats")
        mv = gn.tile([P, KT, 2], F32, tag="mv")
        gst = gn.tile([P, KT, 2], F32, tag="gst")
        for kt in range(KT):
            for sub in range(2):
                nc.vector.bn_stats(stats[:, kt, sub, :], xt[:, kt, sub * 512:(sub + 1) * 512])
            nc.vector.bn_aggr(mv[:, kt, :], stats[:, kt, :, :])
            nc.vector.tensor_copy(gst[:, kt, 0:1], mv[:, kt, 0:1])
            nc.vector.tensor_scalar(
                out=gst[:, kt, 1:2], in0=mv[:, kt, 0:1],
                scalar1=mv[:, kt, 0:1], scalar2=mv[:, kt, 1:2],
                op0=OP.mult, op1=OP.add,
            )
        scale_c = gn.tile([P, KT], F32, tag="scale_c")
        bias_c = gn.tile([P, KT], F32, tag="bias_c")
        for kt in range(KT):
            gsum = pssm.tile([4, 2], F32, tag="pv")
            nc.tensor.matmul(gsum[:], gmask_t[:], gst[:, kt, :], start=True, stop=True)
            gsb = gn.tile([4, 2], F32, tag="gsb")
            nc.vector.tensor_copy(gsb[:], gsum[:])
            gbc = pssm.tile([P, 2], F32, tag="pv")
            nc.tensor.matmul(gbc[:], gmask2_t[:], gsb[:], start=True, stop=True)
            gch = gn.tile([P, 2], F32, tag="gch")
            nc.vector.tensor_copy(gch[:], gbc[:])
            m2 = gn.tile([P, 1], F32, tag="m2")
            nc.vector.tensor_scalar_mul(m2[:], gch[:, 0:1], gch[:, 0:1])
            varg = gn.tile([P, 1], F32, tag="varg")
            nc.vector.tensor_sub(varg[:], gch[:, 1:2], m2[:])
            sd = gn.tile([P, 1], F32, tag="sd")
            nc.scalar.activation(sd[:], varg[:], AF.Sqrt, bias=eps_t[:], scale=1.0)
            rstd = gn.tile([P, 1], F32, tag="rstd")
            nc.vector.reciprocal(rstd[:], sd[:])
            nc.vector.tensor_mul(scale_c[:, kt:kt + 1], rstd[:], gamma_t[:, kt:kt + 1])
            tmp = gn.tile([P, 1], F32, tag="tmp")
            nc.vector.tensor_mul(tmp[:], gch[:, 0:1], scale_c[:, kt:kt + 1])
            nc.vector.tensor_sub(bias_c[:, kt:kt + 1], beta_t[:, kt:kt + 1], tmp[:])
        xn = big.tile([P, KT, N], F32R, tag="xn")
        for kt in range(KT):
            nc.vector.tensor_scalar(
                out=xn[:, kt, :], in0=xt[:, kt, :],
                scalar1=scale_c[:, kt:kt + 1], scalar2=bias_c[:, kt:kt + 1],
                op0=OP.mult, op1=OP.add,
            )
        state.setdefault(b, {})
        state[b]["xt"] = xt
        state[b]["xn"] = xn

    def phase_qkv(b):
        st = state[b]
        xn = st["xn"]
        # ---- q, k projections: [C, N] ----
        qk_sb = []
        for which in range(2):  # 0=q, 1=k
            dst = big.tile([P, KT, N], BF16, tag=f"qk{which}")
            qk_sb.append(dst)
            for m in range(KT):
                ocol = which * C + m * P
                for nf in range(NF):
                    ps = psqp.tile([P, 512], F32, tag="qp")
                    for kt in range(KT):
                        nc.tensor.matmul(
                            ps[:],
                            wqkv_t[:, kt, ocol:ocol + P],
                            xn[:, kt, nf * 512:(nf + 1) * 512],
                            start=(kt == 0), stop=(kt == KT - 1),
                        )
                    dstap = dst[:, m, nf * 512:(nf + 1) * 512]
                    if has_qkv_bias:
                        nc.vector.tensor_scalar_add(dstap, ps[:], bqk_t[:, 2 * which + m:2 * which + m + 1])
                    elif which == 0:
                        nc.vector.tensor_copy(dstap, ps[:])
                    else:
                        nc.scalar.copy(dstap, ps[:])
        st["q"], st["k"] = qk_sb

        # ---- v^T (pixels on partitions): [N, C] + ones columns ----
        vw = 2 * C
        v_aug = big.tile([P, NB, C + 8], BF16, tag="vaug")
        for nb in range(NB):
            ps = pspv.tile([P, C + 8], F32, tag="pv")
            for kt in range(KT):
                nc.tensor.matmul(
                    ps[:, 0:C],
                    xn[:, kt, nb * P:(nb + 1) * P],
                    wqkv_t[:, kt, vw:vw + C],
                    start=(kt == 0), stop=(kt == KT - 1 and not has_qkv_bias),
                )
            if has_qkv_bias:
                nc.tensor.matmul(ps[:, 0:C], ones_col[:], bv_row[:],
                                 start=False, stop=True)
            if nb % 2 == 0:
                nc.vector.tensor_copy(v_aug[:, nb, 0:C], ps[:, 0:C])
            else:
                nc.scalar.copy(v_aug[:, nb, 0:C], ps[:, 0:C])
            nc.vector.tensor_copy(v_aug[:, nb, C:C + 8], one_t[:].to_broadcast([P, 8]))
        st["v"] = v_aug

    def phase_scores(b):
        st = state[b]
        q_sb, k_sb = st["q"], st["k"]
        # ---- S^T = k^T q scaled, exp'd: [m, n] ----
        expS = es.tile([P, NB, N], BF16, tag="expS")
        for mb in range(NB):
            ps = ps512.tile([P, N], F32, tag="mm")
            for nf in range(NF):
                for kt in range(KT):
                    nc.tensor.matmul(
                        ps[:, nf * 512:(nf + 1) * 512],
                        k_sb[:, kt, mb * P:(mb + 1) * P],
                        q_sb[:, kt, nf * 512:(nf + 1) * 512],
                        start=(kt == 0), stop=(kt == KT - 1),
                    )
            nc.scalar.activation(expS[:, mb, :], ps[:], AF.Exp, bias=0.0, scale=1.0 / 16.0)
        st["expS"] = expS

    def pv_group(b, nb):
        # one n-block of att^T = softmax(S) @ V (col C holds row-sums)
        st = state[b]
        expS, v_aug, attT = st["expS"], st["v"], st["attT"]
        ps = pspv.tile([P, C + 8], F32, tag="pv")
        for m in range(NB):
            nc.tensor.matmul(
                ps[:, 0:C + 8],
                expS[:, m, nb * P:(nb + 1) * P],
                v_aug[:, m, 0:C + 8],
                start=(m == 0), stop=(m == NB - 1),
            )
        rinv = gn.tile([P, 1], F32, tag="rinv")
        nc.vector.reciprocal(rinv[:], ps[:, C:C + 1])
        if b == 0:
            # batch 0 normalizes on DVE (ACT is busy with batch 1's exps)
            nc.vector.tensor_scalar_mul(attT[:, nb, :], ps[:, 0:C], rinv[:])
        else:
            # batch 1 normalizes on ACT's free affine (idle post-softmax):
            # out = Copy(in * scale), scale = per-partition rinv
            nc.scalar.activation(attT[:, nb, :], ps[:, 0:C], AF.Copy,
                                 bias=0.0, scale=rinv[:])

    def phase_pv_alloc(b):
        attT = big.tile([P, NB, C], BF16, tag="attT")
        state[b]["attT"] = attT
        att = big.tile([P, KT, N], BF16, tag="att")
        state[b]["att"] = att

    def transpose_group(b, nb, cb):
        # transpose one att^T block -> att [c-block, n-block] via PE
        st = state[b]
        attT, att = st["attT"], st["att"]
        pt = pssm.tile([P, P], BF16, tag="pv")
        nc.tensor.transpose(pt[:], attT[:, nb, cb * P:(cb + 1) * P], ident_t[:])
        if nb % 2 == 0:
            nc.vector.tensor_copy(att[:, cb, nb * P:(nb + 1) * P], pt[:])
        else:
            nc.scalar.copy(att[:, cb, nb * P:(nb + 1) * P], pt[:])

    def proj_group(b, cb, nf):
        # proj + fused (+bias)+residual on the PSUM->SBUF move, then store
        st = state[b]
        att, xt, out_sb = st["att"], st["xt"], st["out_sb"]
        ps = psqp.tile([P, 512], F32, tag="qp")
        for kt in range(KT):
            nc.tensor.matmul(
                ps[:],
                wproj_b[:, kt, cb * P:(cb + 1) * P],
                att[:, kt, nf * 512:(nf + 1) * 512],
                start=(kt == 0), stop=(kt == KT - 1),
            )
        bias_arg = bproj_t[:, cb:cb + 1] if has_proj_bias else 0.0
        nc.vector.scalar_tensor_tensor(
            out=out_sb[:, cb, nf * 512:(nf + 1) * 512],
            in0=ps[:], scalar=bias_arg,
            in1=xt[:, cb, nf * 512:(nf + 1) * 512],
            op0=OP.add, op1=OP.add,
        )
        nc.sync.dma_start(
            out_d[b, cb * P:(cb + 1) * P, nf * 512:(nf + 1) * 512],
            out_sb[:, cb, nf * 512:(nf + 1) * 512])

    # Software-pipelined emission (engines run their streams in order):
    # - both batches' matmul front halves (qkv + scores) go first so the
    #   scalar engine's 33us of exp work runs back-to-back,
    # - batch 0's PV as soon as its exps land,
    # - batch 0's transposes interleave with batch 1's PV groups,
    # - batch 1's transposes interleave with batch 0's proj/store,
    # keeping PE dense through the latency-bound transpose chains.
    assert BPC == 2
    phase_load_gn(0)
    phase_qkv(0)
    phase_scores(0)
    phase_load_gn(1)
    phase_qkv(1)
    phase_scores(1)
    for b in range(BPC):
        out_sb = big.tile([P, KT, N], F32, tag="outsb")
        state[b]["out_sb"] = out_sb
        phase_pv_alloc(b)
    # batch 0's PV/transpose/proj run in the PE bubble while ACT still
    # drains batch 1's exps; batch 1's tail follows immediately after.
    for b in range(BPC):
        for nb in range(NB):
            pv_group(b, nb)
        for nb in range(NB):
            for cb in range(KT):
                transpose_group(b, nb, cb)
        for cb in range(KT):
            for nf in range(NF):
                proj_group(b, cb, nf)


_BUILD_CACHE = {}


def _build(has_qkv_bias: bool, has_proj_bias: bool, stop_stage: int = 99):
    key = (has_qkv_bias, has_proj_bias, stop_stage)
    if key in _BUILD_CACHE:
        return _BUILD_CACHE[key]
    nc = bacc.Bacc(
        "TRN2", target_bir_lowering=False, debug=False, enable_asserts=False
    )
    x_d = nc.dram_tensor("x", [BPC, C, N], F32, kind="ExternalInput")
    gamma_d = nc.dram_tensor("gamma", [C, 1], F32, kind="ExternalInput")
    beta_d = nc.dram_tensor("beta", [C, 1], F32, kind="ExternalInput")
    wqkvT_d = nc.dram_tensor("w_qkvT", [C, 3 * C], F32, kind="ExternalInput")
    bqkv_d = nc.dram_tensor("b_qkv", [3 * C, 1], F32, kind="ExternalInput")
    wprojT_d = nc.dram_tensor("w_projT", [C, C], F32, kind="ExternalInput")
    bproj_d = nc.dram_tensor("b_proj", [C, 1], F32, kind="ExternalInput")
    out_d = nc.dram_tensor("out", [BPC, C, N], F32, kind="ExternalOutput")

    ident_np = np.eye(P, dtype=np.float32)
    gmask_np = np.zeros((P, 4), dtype=np.float32)
    for c in range(P):
        gmask_np[c, c // GS] = 1.0 / GS
    gmask2_np = np.zeros((4, P), dtype=np.float32)
    for c in range(P):
        gmask2_np[c // GS, c] = 1.0
    ident_d = nc.inline_tensor(ident_np, "ident")
    gmask_d = nc.inline_tensor(gmask_np, "gmask")
    gmask2_d = nc.inline_tensor(gmask2_np, "gmask2")

    with tile.TileContext(nc) as tc:
        attn_tile_kernel(
            tc, out_d, x_d, gamma_d, beta_d, wqkvT_d, bqkv_d, wprojT_d,
            bproj_d, ident_d, gmask_d, gmask2_d, has_qkv_bias, has_proj_bias,
            stop_stage=stop_stage,
        )
    nc.compile()
    _BUILD_CACHE[key] = nc
    return nc


def kernel(**inputs) -> np.ndarray:
    x = np.ascontiguousarray(np.asarray(inputs["x"], dtype=np.float32))
    gamma = np.asarray(inputs["gamma"], np.float32).reshape(C, 1)
    beta = np.asarray(inputs["beta"], np.float32).reshape(C, 1)
    w_qkv = np.asarray(inputs["w_qkv"], np.float32)
    b_qkv = np.asarray(inputs["b_qkv"], np.float32).reshape(3 * C, 1)
    w_proj = np.asarray(inputs["w_proj"], np.float32)
    b_proj = np.asarray(inputs["b_proj"], np.float32).reshape(C, 1)

    wqkvT = np.ascontiguousarray(w_qkv.T)    # [C, 3C]
    wprojT = np.ascontiguousarray(w_proj.T)  # [C, C]
    has_qkv_bias = bool(np.any(b_qkv))
    has_proj_bias = bool(np.any(b_proj))

    nc = _build(has_qkv_bias, has_proj_bias)

    shared = {
        "gamma": np.ascontiguousarray(gamma),
        "beta": np.ascontiguousarray(beta),
        "w_qkvT": wqkvT,
        "b_qkv": np.ascontiguousarray(b_qkv),
        "w_projT": wprojT,
        "b_proj": np.ascontiguousarray(b_proj),
    }
    in_maps = []
    for core in range(NCORES):
        xm = np.ascontiguousarray(
            x[core * BPC:(core + 1) * BPC].reshape(BPC, C, N)
        )
        in_maps.append({"x": xm, **shared})

    res = run_bass_kernel_spmd(nc, in_maps, core_ids=list(range(NCORES)))
    out = np.concatenate(
        [r["out"].reshape(BPC, C, H, W) for r in res.results], axis=0
    )
    return np.ascontiguousarray(out.astype(np.float32))



# revision 6
# speedup vs baseline: 1.1760x; 1.1760x over previous
"""AttentionBlock Trainium2 kernel (v2: fp8 DoubleRow attention core).

Problem: B=16, C=256, H=W=32 (N=1024 pixels), GroupNorm(8) -> 1x1-conv QKV ->
softmax attention over pixels -> 1x1-conv proj -> +residual.

Sharding: data-parallel over batch across 8 NeuronCores (2 batch elems/core),
weights replicated.  Weights are pre-transposed on the host so every matmul
operand DMAs contiguously.

Device layout per batch element (channels on partitions):
  x                  : [C=256, N=1024] fp32 (residual + groupnorm stats)
  xn                 : [C, N] bf16 (full-rate PE matmuls, fast LDWEIGHTS)
  q, k               : [C, N] fp8e4 -> scores via one DoubleRow matmul per
                       (mb, nf): contraction C=256 packed 2/partition at
                       0.5 cycles/row
  v_aug              : [N, C+8] fp8e4 (pixels on partitions; cols 256.. = ones
                       so the P@V matmul also produces the softmax row-sums)
  expS = exp(S/16-c) : [m, n] fp8e4, exp'd by ScalarE straight out of PSUM
                       (constant logit shift cancels in the normalize;
                       max logit ~5.6 -> exp in fp8e4 range, max 240)
  att^T = P@V        : 4 DoubleRow matmuls per n-block (m packed in pairs),
                       then PE-transposed to [C, n] bf16 for the proj.
Residual + proj bias are fused into the PSUM->SBUF move with one
scalar_tensor_tensor op, keeping the residual exact fp32.

ACT-table hygiene: both batches' GroupNorm (Sqrt) run before any Exp, and
mid-stream drains stay on DVE, so the activation table switches only a
couple of times instead of thrashing Sqrt/Exp/Copy.
"""

from contextlib import ExitStack

import numpy as np

import concourse.bass as bass
import concourse.tile as tile
from concourse import bacc, mybir
from concourse._compat import with_exitstack
from concourse.bass_utils import run_bass_kernel_spmd

# Problem constants (hardcoded per contract)
B, C, H, W = 16, 256, 32, 32
N = H * W            # 1024 pixels
G = 8                # groups
GS = C // G          # 32 channels / group
NCORES = 8
BPC = B // NCORES    # batch elems per core
EPS = 1e-5
P = 128              # partitions
KT = C // P          # 2 c-tiles
NB = N // P          # 8 pixel blocks of 128
NF = N // 512        # 2 free chunks of 512
EXP_SHIFT = -1.5     # constant logit shift; cancels in softmax normalize
F32 = mybir.dt.float32
BF16 = mybir.dt.bfloat16
FP8 = mybir.dt.float8e4
AF = mybir.ActivationFunctionType
OP = mybir.AluOpType
DR = mybir.MatmulPerfMode.DoubleRow


@with_exitstack
def attn_tile_kernel(
    ctx: ExitStack,
    tc: tile.TileContext,
    out_d,
    x_d,
    gamma_d,
    beta_d,
    wqkvT_d,
    bqkv_d,
    wprojT_d,
    bproj_d,
    ident_d,
    gmask_d,
    gmask2_d,
    has_qkv_bias: bool,
    has_proj_bias: bool,
):
    nc = tc.nc

    consts = ctx.enter_context(tc.tile_pool(name="consts", bufs=1))
    gn = ctx.enter_context(tc.tile_pool(name="gn", bufs=2))
    big = ctx.enter_context(tc.tile_pool(name="big", bufs=2))
    es = ctx.enter_context(tc.tile_pool(name="es", bufs=2))
    # PSUM budget (8 banks), split by draining engine so phases don't
    # block each other on slots:
    #   mm  = 2x [128,1024] S-score psums, drained by ACT exp  (4 banks)
    #   qp  = 2x [128,512] qkv/proj psums, drained by DVE      (2 banks)
    #   pv  = 2x [128,264] v/PV accum + transposes + gn stats  (2 banks)
    ps512 = ctx.enter_context(tc.tile_pool(name="ps512", bufs=2, space="PSUM"))
    psqp = ctx.enter_context(tc.tile_pool(name="psqp", bufs=2, space="PSUM"))
    pspv = ctx.enter_context(tc.tile_pool(name="pspv", bufs=2, space="PSUM"))
    pssm = pspv

    state = {}

    # batch inputs go first: they gate the groupnorm -> QKV critical path,
    # while weights aren't needed until the first matmul ~8us later
    for b in range(BPC):
        xt = big.tile([P, KT, N], F32, tag="xt")
        for kt in range(KT):
            for sub in range(2):
                nc.sync.dma_start(
                    xt[:, kt, sub * 512:(sub + 1) * 512],
                    x_d[b, kt * P:(kt + 1) * P, sub * 512:(sub + 1) * 512])
        state[b] = {"xt": xt}

    # ---- constants / weights (loaded once, replicated across cores) ----
    gmask_t = consts.tile([P, 4], F32)
    nc.sync.dma_start(gmask_t[:], gmask_d.ap())
    gmask2_t = consts.tile([4, P], F32)
    nc.sync.dma_start(gmask2_t[:], gmask2_d.ap())
    gamma_t = consts.tile([P, KT], F32)
    beta_t = consts.tile([P, KT], F32)
    for kt in range(KT):
        nc.sync.dma_start(gamma_t[:, kt:kt + 1], gamma_d[kt * P:(kt + 1) * P, :])
        nc.sync.dma_start(beta_t[:, kt:kt + 1], beta_d[kt * P:(kt + 1) * P, :])
    wqkv_f = consts.tile([P, KT, 3 * C], F32)
    for kt in range(KT):
        nc.sync.dma_start(wqkv_f[:, kt, :], wqkvT_d[kt * P:(kt + 1) * P, :])
    wqkv_t = consts.tile([P, KT, 3 * C], BF16)
    for kt in range(KT):
        nc.vector.tensor_copy(wqkv_t[:, kt, :], wqkv_f[:, kt, :])
    wproj_t = consts.tile([P, KT, C], F32)
    for kt in range(KT):
        nc.sync.dma_start(wproj_t[:, kt, :], wprojT_d[kt * P:(kt + 1) * P, :])
    wproj_b = consts.tile([P, KT, C], BF16)
    for kt in range(KT):
        nc.vector.tensor_copy(wproj_b[:, kt, :], wproj_t[:, kt, :])
    ident_f = consts.tile([P, P], F32)
    nc.sync.dma_start(ident_f[:], ident_d.ap())
    ident_t = consts.tile([P, P], BF16)
    nc.vector.tensor_copy(ident_t[:], ident_f[:])
    eps_t = consts.tile([P, 1], F32)
    nc.vector.memset(eps_t, EPS)
    shift_t = consts.tile([P, 1], F32)
    nc.vector.memset(shift_t, EXP_SHIFT)
    # q/k biases as per-partition columns (q: rows 0..255, k: 256..511)
    if has_qkv_bias:
        bqk_t = consts.tile([P, 2 * KT], F32)
        for j in range(2 * KT):
            nc.sync.dma_start(bqk_t[:, j:j + 1], bqkv_d[j * P:(j + 1) * P, :])
        # v bias lives on the free axis of v_aug -> needs a row layout + ones col
        bv_row = consts.tile([1, C], F32)
        nc.sync.dma_start(
            bv_row[:], bqkv_d[2 * C:3 * C, :].rearrange("c one -> one c"))
        ones_row_f32 = consts.tile([1, P], F32)
        nc.vector.memset(ones_row_f32, 1.0)
    if has_proj_bias:
        bproj_t = consts.tile([P, KT], F32)
        for kt in range(KT):
            nc.sync.dma_start(bproj_t[:, kt:kt + 1], bproj_d[kt * P:(kt + 1) * P, :])

    def phase_load_gn(b):
        # ---- group norm stats -> per-channel scale/bias -> xn (bf16) ----
        st = state[b]
        xt = st["xt"]
        stats = gn.tile([P, KT, 2, 6], F32, tag="stats")
        mv = gn.tile([P, KT, 2], F32, tag="mv")
        gst = gn.tile([P, KT, 2], F32, tag="gst")
        for kt in range(KT):
            for sub in range(2):
                nc.vector.bn_stats(stats[:, kt, sub, :], xt[:, kt, sub * 512:(sub + 1) * 512])
            nc.vector.bn_aggr(mv[:, kt, :], stats[:, kt, :, :])
            nc.vector.tensor_copy(gst[:, kt, 0:1], mv[:, kt, 0:1])
            nc.vector.tensor_scalar(
                out=gst[:, kt, 1:2], in0=mv[:, kt, 0:1],
                scalar1=mv[:, kt, 0:1], scalar2=mv[:, kt, 1:2],
                op0=OP.mult, op1=OP.add,
            )
        scale_c = gn.tile([P, KT], F32, tag="scale_c")
        bias_c = gn.tile([P, KT], F32, tag="bias_c")
        for kt in range(KT):
            gsum = pssm.tile([4, 2], F32, tag="pv")
            nc.tensor.matmul(gsum[:], gmask_t[:], gst[:, kt, :], start=True, stop=True)
            gsb = gn.tile([4, 2], F32, tag="gsb")
            nc.vector.tensor_copy(gsb[:], gsum[:])
            gbc = pssm.tile([P, 2], F32, tag="pv")
            nc.tensor.matmul(gbc[:], gmask2_t[:], gsb[:], start=True, stop=True)
            gch = gn.tile([P, 2], F32, tag="gch")
            nc.vector.tensor_copy(gch[:], gbc[:])
            m2 = gn.tile([P, 1], F32, tag="m2")
            nc.vector.tensor_scalar_mul(m2[:], gch[:, 0:1], gch[:, 0:1])
            varg = gn.tile([P, 1], F32, tag="varg")
            nc.vector.tensor_sub(varg[:], gch[:, 1:2], m2[:])
            sd = gn.tile([P, 1], F32, tag="sd")
            nc.scalar.activation(sd[:], varg[:], AF.Sqrt, bias=eps_t[:], scale=1.0)
            rstd = gn.tile([P, 1], F32, tag="rstd")
            nc.vector.reciprocal(rstd[:], sd[:])
            nc.vector.tensor_mul(scale_c[:, kt:kt + 1], rstd[:], gamma_t[:, kt:kt + 1])
            tmp = gn.tile([P, 1], F32, tag="tmp")
            nc.vector.tensor_mul(tmp[:], gch[:, 0:1], scale_c[:, kt:kt + 1])
            nc.vector.tensor_sub(bias_c[:, kt:kt + 1], beta_t[:, kt:kt + 1], tmp[:])
        xn = big.tile([P, KT, N], BF16, tag="xn")
        for kt in range(KT):
            nc.vector.tensor_scalar(
                out=xn[:, kt, :], in0=xt[:, kt, :],
                scalar1=scale_c[:, kt:kt + 1], scalar2=bias_c[:, kt:kt + 1],
                op0=OP.mult, op1=OP.add,
            )
        st["xn"] = xn

    def phase_qkv(b):
        st = state[b]
        xn = st["xn"]
        # ---- q, k projections -> fp8 [C(kt-paired), N] ----
        qk_sb = []
        for which in range(2):  # 0=q, 1=k
            dst = big.tile([P, KT, N], FP8, tag=f"qk{which}")
            qk_sb.append(dst)
            for m in range(KT):
                ocol = which * C + m * P
                pss = [psqp.tile([P, 512], F32, tag="qp", name=f"qkps{nf}")
                       for nf in range(NF)]
                # kt-outer so the stationary w-block is reused across nf
                for kt in range(KT):
                    for nf in range(NF):
                        nc.tensor.matmul(
                            pss[nf][:],
                            wqkv_t[:, kt, ocol:ocol + P],
                            xn[:, kt, nf * 512:(nf + 1) * 512],
                            start=(kt == 0), stop=(kt == KT - 1),
                        )
                for nf in range(NF):
                    dstap = dst[:, m, nf * 512:(nf + 1) * 512]
                    if has_qkv_bias:
                        nc.vector.tensor_scalar_add(
                            dstap, pss[nf][:],
                            bqk_t[:, 2 * which + m:2 * which + m + 1])
                    else:
                        nc.vector.tensor_copy(dstap, pss[nf][:])
        st["q"], st["k"] = qk_sb

        # ---- v^T (pixels on partitions): fp8 [N, C] + ones columns ----
        vw = 2 * C
        v_aug = big.tile([P, NB, C + 8], FP8, tag="vaug")
        for nb in range(NB):
            ps = pspv.tile([P, C + 8], F32, tag="pv")
            for kt in range(KT):
                nc.tensor.matmul(
                    ps[:, 0:C],
                    xn[:, kt, nb * P:(nb + 1) * P],
                    wqkv_t[:, kt, vw:vw + C],
                    start=(kt == 0), stop=(kt == KT - 1 and not has_qkv_bias),
                )
            if has_qkv_bias:
                nc.tensor.matmul(ps[:, 0:C], ones_row_f32[:], bv_row[:],
                                 start=False, stop=True)
            nc.vector.tensor_copy(v_aug[:, nb, 0:C], ps[:, 0:C])
            nc.vector.memset(v_aug[:, nb, C:C + 8], 1.0)
        st["v"] = v_aug

    def phase_scores(b):
        st = state[b]
        q_sb, k_sb = st["q"], st["k"]
        # ---- S^T = k^T q, exp'd -> fp8 [m, n]; one DoubleRow matmul per
        # (mb, nf): both c-tiles contracted in a single pass ----
        expS = es.tile([P, NB, N], FP8, tag="expS")
        for mb in range(NB):
            ps = ps512.tile([P, N], F32, tag="mm")
            for nf in range(NF):
                nc.tensor.matmul(
                    ps[:, nf * 512:(nf + 1) * 512],
                    k_sb[:, :, mb * P:(mb + 1) * P],
                    q_sb[:, :, nf * 512:(nf + 1) * 512],
                    start=True, stop=True, perf_mode=DR,
                )
            nc.scalar.activation(expS[:, mb, :], ps[:], AF.Exp,
                                 bias=shift_t[:], scale=1.0 / 16.0)
        st["expS"] = expS

    def pv_group(b, nb):
        # one n-block of att^T = softmax(S) @ V (col C holds row-sums);
        # m contracted in 4 DoubleRow pairs
        st = state[b]
        expS, v_aug, attT = st["expS"], st["v"], st["attT"]
        ps = pspv.tile([P, C + 8], F32, tag="pv")
        for j in range(NB // 2):
            nc.tensor.matmul(
                ps[:, 0:C + 8],
                expS[:, 2 * j:2 * j + 2, nb * P:(nb + 1) * P],
                v_aug[:, 2 * j:2 * j + 2, 0:C + 8],
                start=(j == 0), stop=(j == NB // 2 - 1), perf_mode=DR,
            )
        rinv = gn.tile([P, 1], F32, tag="rinv")
        nc.vector.reciprocal(rinv[:], ps[:, C:C + 1])
        if b == 0:
            # batch 0 normalizes on DVE (ACT may still be draining exps)
            nc.vector.tensor_scalar_mul(attT[:, nb, :], ps[:, 0:C], rinv[:])
        else:
            # batch 1 normalizes on ACT's free affine (idle post-softmax):
            # out = Copy(in * scale), scale = per-partition rinv
            nc.scalar.activation(attT[:, nb, :], ps[:, 0:C], AF.Copy,
                                 bias=0.0, scale=rinv[:])

    def phase_pv_alloc(b):
        attT = big.tile([P, NB, C], BF16, tag="attT")
        state[b]["attT"] = attT
        att = big.tile([P, KT, N], BF16, tag="att")
        state[b]["att"] = att

    def transpose_group(b, nb, cb):
        # transpose one att^T block -> att [c-block, n-block] via PE
        st = state[b]
        attT, att = st["attT"], st["att"]
        pt = pssm.tile([P, P], BF16, tag="pv")
        nc.tensor.transpose(pt[:], attT[:, nb, cb * P:(cb + 1) * P], ident_t[:])
        if b == 0:
            nc.vector.tensor_copy(att[:, cb, nb * P:(nb + 1) * P], pt[:])
        else:
            nc.scalar.copy(att[:, cb, nb * P:(nb + 1) * P], pt[:])

    def proj_group(b, cb):
        # proj + fused (+bias)+residual on the PSUM->SBUF move, then store
        st = state[b]
        att, xt, out_sb = st["att"], st["xt"], st["out_sb"]
        pss = [psqp.tile([P, 512], F32, tag="qp", name=f"projps{nf}")
               for nf in range(NF)]
        # kt-outer so the stationary wproj-block is reused across nf
        for kt in range(KT):
            for nf in range(NF):
                nc.tensor.matmul(
                    pss[nf][:],
                    wproj_b[:, kt, cb * P:(cb + 1) * P],
                    att[:, kt, nf * 512:(nf + 1) * 512],
                    start=(kt == 0), stop=(kt == KT - 1),
                )
        bias_arg = bproj_t[:, cb:cb + 1] if has_proj_bias else 0.0
        for nf in range(NF):
            nc.vector.scalar_tensor_tensor(
                out=out_sb[:, cb, nf * 512:(nf + 1) * 512],
                in0=pss[nf][:], scalar=bias_arg,
                in1=xt[:, cb, nf * 512:(nf + 1) * 512],
                op0=OP.add, op1=OP.add,
            )
            nc.sync.dma_start(
                out_d[b, cb * P:(cb + 1) * P, nf * 512:(nf + 1) * 512],
                out_sb[:, cb, nf * 512:(nf + 1) * 512])

    # Software-pipelined emission (engines run their streams in order):
    # - both groupnorms first so ACT's Sqrt ops all precede the first Exp
    #   (no activation-table thrash mid-stream),
    # - both batches' matmul front halves (qkv + scores) next so ACT's
    #   exp work runs back-to-back,
    # - batch 0's PV/transpose/proj run in the PE bubble while ACT still
    #   drains batch 1's exps; batch 1's tail follows immediately after.
    assert BPC == 2
    phase_load_gn(0)
    phase_load_gn(1)
    phase_qkv(0)
    phase_scores(0)
    phase_qkv(1)
    phase_scores(1)
    for b in range(BPC):
        out_sb = big.tile([P, KT, N], F32, tag="outsb")
        state[b]["out_sb"] = out_sb
        phase_pv_alloc(b)
    for b in range(BPC):
        for nb in range(NB):
            pv_group(b, nb)
        for nb in range(NB):
            for cb in range(KT):
                transpose_group(b, nb, cb)
        for cb in range(KT):
            proj_group(b, cb)


_BUILD_CACHE = {}


def _build(has_qkv_bias: bool, has_proj_bias: bool):
    key = (has_qkv_bias, has_proj_bias)
    if key in _BUILD_CACHE:
        return _BUILD_CACHE[key]
    nc = bacc.Bacc(
        "TRN2", target_bir_lowering=False, debug=False, enable_asserts=False
    )
    x_d = nc.dram_tensor("x", [BPC, C, N], F32, kind="ExternalInput")
    gamma_d = nc.dram_tensor("gamma", [C, 1], F32, kind="ExternalInput")
    beta_d = nc.dram_tensor("beta", [C, 1], F32, kind="ExternalInput")
    wqkvT_d = nc.dram_tensor("w_qkvT", [C, 3 * C], F32, kind="ExternalInput")
    bqkv_d = nc.dram_tensor("b_qkv", [3 * C, 1], F32, kind="ExternalInput")
    wprojT_d = nc.dram_tensor("w_projT", [C, C], F32, kind="ExternalInput")
    bproj_d = nc.dram_tensor("b_proj", [C, 1], F32, kind="ExternalInput")
    out_d = nc.dram_tensor("out", [BPC, C, N], F32, kind="ExternalOutput")

    ident_np = np.eye(P, dtype=np.float32)
    gmask_np = np.zeros((P, 4), dtype=np.float32)
    for c in range(P):
        gmask_np[c, c // GS] = 1.0 / GS
    gmask2_np = np.zeros((4, P), dtype=np.float32)
    for c in range(P):
        gmask2_np[c // GS, c] = 1.0
    ident_d = nc.inline_tensor(ident_np, "ident")
    gmask_d = nc.inline_tensor(gmask_np, "gmask")
    gmask2_d = nc.inline_tensor(gmask2_np, "gmask2")

    with tile.TileContext(nc) as tc:
        attn_tile_kernel(
            tc, out_d, x_d, gamma_d, beta_d, wqkvT_d, bqkv_d, wprojT_d,
            bproj_d, ident_d, gmask_d, gmask2_d, has_qkv_bias, has_proj_bias,
        )
    nc.compile()
    _BUILD_CACHE[key] = nc
    return nc


def kernel(**inputs) -> np.ndarray:
    x = np.ascontiguousarray(np.asarray(inputs["x"], dtype=np.float32))
    gamma = np.asarray(inputs["gamma"], np.float32).reshape(C, 1)
    beta = np.asarray(inputs["beta"], np.float32).reshape(C, 1)
    w_qkv = np.asarray(inputs["w_qkv"], np.float32)
    b_qkv = np.asarray(inputs["b_qkv"], np.float32).reshape(3 * C, 1)
    w_proj = np.asarray(inputs["w_proj"], np.float32)
    b_proj = np.asarray(inputs["b_proj"], np.float32).reshape(C, 1)

    wqkvT = np.ascontiguousarray(w_qkv.T)    # [C, 3C]
    wprojT = np.ascontiguousarray(w_proj.T)  # [C, C]
    has_qkv_bias = bool(np.any(b_qkv))
    has_proj_bias = bool(np.any(b_proj))

    nc = _build(has_qkv_bias, has_proj_bias)

    shared = {
        "gamma": np.ascontiguousarray(gamma),
        "beta": np.ascontiguousarray(beta),
        "w_qkvT": wqkvT,
        "b_qkv": np.ascontiguousarray(b_qkv),
        "w_projT": wprojT,
        "b_proj": np.ascontiguousarray(b_proj),
    }
    in_maps = []
    for core in range(NCORES):
        xm = np.ascontiguousarray(
            x[core * BPC:(core + 1) * BPC].reshape(BPC, C, N)
        )
        in_maps.append({"x": xm, **shared})

    res = run_bass_kernel_spmd(nc, in_maps, core_ids=list(range(NCORES)))
    out = np.concatenate(
        [r["out"].reshape(BPC, C, H, W) for r in res.results], axis=0
    )
    return np.ascontiguousarray(out.astype(np.float32))
